# revision 33
# baseline (speedup 1.0000x reference)
"""GAT attention-score kernel for Trainium2 (8 NeuronCores, SPMD).

Computes e = LeakyReLU(Wx_i @ a[:D] + Wx_j @ a[D:], slope=0.2) for
E=640000 edges, D=128, sharded over 8 cores along the edge dimension
(a is replicated to every core).

Per-core layout (E_CORE = 80000 edges):
  - partition p owns edges [p*EPP, (p+1)*EPP) of the core's shard
  - T tiles of C edges/partition; each tile is one SBUF tensor
    [128, 2, C, 128]; Wx_i loads go on the SP HWDGE ring and Wx_j loads
    on the ACT HWDGE ring (dual-ring: measured 319 -> 390 GB/s), with
    loads emitted LA tiles ahead of compute.  The attention vector a is
    broadcast once to all partitions (a_sb [128, 2, 128]).
  - each tile's C edges split into three chains chosen so the two slow
    engines (GPSIMD, ScalarE) never gate each other:
      chain 1, edges [0, Q1):        VectorE mult -> VectorE reduce
      chain 2, edges [Q1, Q1+Q2):    VectorE mult -> ScalarE accums
      chain 3, edges [Q1+Q2, C):     GPSIMD mult -> VectorE reduce
    (the VectorE mult for chains 1+2 is a single instruction over
    edges [0, Q1+Q2))
  - LeakyReLU (ScalarE Prelu, alpha=0.2) per chain into a [128, EPP]
    result buffer; one store DMA at the end.
"""

import sys

if "/opt/trn_rl_repo" not in sys.path:
    sys.path.insert(0, "/opt/trn_rl_repo")

from contextlib import ExitStack

import numpy as np

import concourse.bass as bass
import concourse.bacc as bacc
import concourse.mybir as mybir
import concourse.tile as tile
from concourse.bass_utils import run_bass_kernel_spmd

N_CORES = 8
E = 640000
D = 128
REC = 2 * D
E_CORE = E // N_CORES  # 80000
P = 128
EPP = E_CORE // P  # 625 edges per partition
NEG_SLOPE = 0.2
F32 = mybir.dt.float32
MULT = mybir.AluOpType.mult
ADD = mybir.AluOpType.add


def _bcast_free(ap: bass.AP, count: int, axis: int) -> bass.AP:
    """Insert a stride-0 free dim of `count` at free-axis position `axis`."""
    dims = list(ap.ap)
    dims.insert(1 + axis, [0, count])
    return bass.AP(tensor=ap.tensor, offset=ap.offset, ap=dims)


def build_program(
    epp: int = EPP,
    c: int = 25,
    bufs: int = 5,
    q1: int = 7,
    q2: int = 9,
    la: int = 3,
) -> bass.Bass:
    """Build the per-core Bass program for `epp` edges per partition."""
    assert epp % c == 0 and q1 + q2 < c
    t_tiles = epp // c
    e_core = P * epp
    q3 = c - q1 - q2
    q12 = q1 + q2

    nc = bacc.Bacc()
    wi_d = nc.dram_tensor("Wx_i", [e_core, D], F32, kind="ExternalInput")
    wj_d = nc.dram_tensor("Wx_j", [e_core, D], F32, kind="ExternalInput")
    a_d = nc.dram_tensor("a", [REC], F32, kind="ExternalInput")
    out_d = nc.dram_tensor("out", [e_core], F32, kind="ExternalOutput")

    wi = wi_d[:].rearrange("(p n) d -> p n d", p=P)  # [128, epp, 128]
    wj = wj_d[:].rearrange("(p n) d -> p n d", p=P)
    out_r = out_d[:].rearrange("(p n) -> p n", p=P)  # [128, epp]

    with tile.TileContext(nc) as tc, ExitStack() as ctx:
        const_pool = ctx.enter_context(tc.tile_pool(name="const", bufs=1))
        in_pool = ctx.enter_context(tc.tile_pool(name="inp", bufs=bufs))
        pv_pool = ctx.enter_context(tc.tile_pool(name="pv", bufs=2))
        pg_pool = ctx.enter_context(tc.tile_pool(name="pg", bufs=2))
        acc_pool = ctx.enter_context(tc.tile_pool(name="acc", bufs=6))
        res_pool = ctx.enter_context(tc.tile_pool(name="res", bufs=1))

        # attention vector broadcast to all 128 partitions: [128, 2, 128]
        a_sb = const_pool.tile([P, 2, D], F32)
        a_ap = a_d[:]
        a_bcast = bass.AP(
            tensor=a_ap.tensor, offset=a_ap.offset, ap=[[0, P]] + list(a_ap.ap)
        )
        nc.gpsimd.dma_start(out=a_sb[:].rearrange("p a d -> p (a d)"), in_=a_bcast)

        # stride-0 garbage sink for the elementwise out of ScalarE accums
        sink_s = const_pool.tile([P, 1], F32)

        res = res_pool.tile([P, epp], F32)

        # loads are emitted `la` tiles ahead of compute
        recs: dict[int, object] = {}

        def emit_loads(t: int) -> None:
            rec = in_pool.tile([P, 2, c, D], F32, tag="rec")
            recs[t] = rec
            nc.sync.dma_start(out=rec[:, 0, :, :], in_=wi[:, t * c : (t + 1) * c, :])
            nc.scalar.dma_start(out=rec[:, 1, :, :], in_=wj[:, t * c : (t + 1) * c, :])

        for t in range(min(la, t_tiles)):
            emit_loads(t)

        for t in range(t_tiles):
            if t + la < t_tiles:
                emit_loads(t + la)
            rec = recs.pop(t)

            # ---- VectorE mult for chains 1+2 (edges [0, q12))
            prod_v = pv_pool.tile([P, 2, q12, D], F32, tag="pv")
            nc.vector.tensor_tensor(
                out=prod_v[:],
                in0=rec[:, :, 0:q12, :],
                in1=_bcast_free(a_sb[:], q12, axis=1),
                op=MULT,
            )
            # ---- GPSIMD mult for chain 3 (edges [q12, c))
            prod_g = pg_pool.tile([P, 2, q3, D], F32, tag="pg")
            nc.gpsimd.tensor_tensor(
                out=prod_g[:],
                in0=rec[:, :, q12:c, :],
                in1=_bcast_free(a_sb[:], q3, axis=1),
                op=MULT,
            )

            # ---- chain 1 reduce on VectorE
            acc_v = acc_pool.tile([P, q1], F32, tag="acc_v")
            nc.vector.tensor_reduce(
                out=acc_v[:],
                in_=prod_v[:, :, 0:q1, :].rearrange("p m c d -> p c m d"),
                axis=mybir.AxisListType.XY,
                op=ADD,
            )
            # ---- chain 3 reduce on VectorE
            acc_g = acc_pool.tile([P, q3], F32, tag="acc_g")
            nc.vector.tensor_reduce(
                out=acc_g[:],
                in_=prod_g[:].rearrange("p m c d -> p c m d"),
                axis=mybir.AxisListType.XY,
                op=ADD,
            )

            # ---- chain 2: ScalarE accumulates per edge from prod_v
            acc_a = acc_pool.tile([P, q2], F32, tag="acc_a")
            for cc in range(q2):
                in_ = prod_v[:, :, q1 + cc, :]
                nc.scalar.activation(
                    out=sink_s[:].broadcast_to(in_.shape),
                    in_=in_,
                    func=mybir.ActivationFunctionType.Copy,
                    accum_out=acc_a[:, cc : cc + 1],
                )

            # leaky relus on ScalarE into the result buffer
            base = t * c
            for accx, lo, hi in (
                (acc_v, 0, q1),
                (acc_a, q1, q12),
                (acc_g, q12, c),
            ):
                nc.scalar.activation(
                    out=res[:, base + lo : base + hi],
                    in_=accx[:],
                    func=mybir.ActivationFunctionType.Prelu,
                    alpha=NEG_SLOPE,
                )

        nc.scalar.dma_start(out=out_r[:, :], in_=res[:])

    nc.compile()
    return nc


_CACHED_NC = None


def kernel(Wx_i: np.ndarray, Wx_j: np.ndarray, a: np.ndarray) -> np.ndarray:
    global _CACHED_NC
    if _CACHED_NC is None:
        _CACHED_NC = build_program()
    nc = _CACHED_NC

    Wx_i = np.ascontiguousarray(np.asarray(Wx_i, dtype=np.float32))
    Wx_j = np.ascontiguousarray(np.asarray(Wx_j, dtype=np.float32))
    a = np.ascontiguousarray(np.asarray(a, dtype=np.float32))

    in_maps = []
    for i in range(N_CORES):
        sl = slice(i * E_CORE, (i + 1) * E_CORE)
        in_maps.append(
            {
                "Wx_i": np.ascontiguousarray(Wx_i[sl]),
                "Wx_j": np.ascontiguousarray(Wx_j[sl]),
                "a": a,
            }
        )

    r = run_bass_kernel_spmd(nc, in_maps, core_ids=list(range(N_CORES)))
    return np.concatenate([m["out"] for m in r.results])


# revision 34
# speedup vs baseline: 1.5034x; 1.5034x over previous
"""GAT attention-score kernel for Trainium2 (8 NeuronCores, SPMD).

Computes e = LeakyReLU(Wx_i @ a[:D] + Wx_j @ a[D:], slope=0.2) for
E=640000 edges, D=128, sharded over 8 cores along the edge dimension
(a is replicated to every core).

Per-core layout (E_CORE = 80000 edges):
  - partition p owns edges [p*EPP, (p+1)*EPP) of the core's shard
  - T tiles of C edges/partition; each tile is one SBUF tensor
    [128, 2, C, 128]; Wx_i loads go on the SP HWDGE ring and Wx_j loads
    on the ACT HWDGE ring (dual-ring: measured 319 -> 390 GB/s), with
    loads emitted LA tiles ahead of compute.  The attention vector a is
    broadcast once to all partitions (a_sb [128, 2, 128]).
  - each tile's C edges split into three chains chosen so the two slow
    engines (GPSIMD, ScalarE) never gate each other:
      chain 1, edges [0, Q1):        VectorE mult -> VectorE reduce
      chain 2, edges [Q1, Q1+Q2):    VectorE mult -> ScalarE accums
      chain 3, edges [Q1+Q2, C):     GPSIMD mult -> VectorE reduce
    (the VectorE mult for chains 1+2 is a single instruction over
    edges [0, Q1+Q2))
  - LeakyReLU (ScalarE Prelu, alpha=0.2) per chain into a [128, EPP]
    result buffer; one store DMA at the end.
"""

import sys

if "/opt/trn_rl_repo" not in sys.path:
    sys.path.insert(0, "/opt/trn_rl_repo")

from contextlib import ExitStack

import numpy as np

import concourse.bass as bass
import concourse.bacc as bacc
import concourse.mybir as mybir
import concourse.tile as tile
from concourse.bass_utils import run_bass_kernel_spmd

N_CORES = 8
E = 640000
D = 128
REC = 2 * D
E_CORE = E // N_CORES  # 80000
P = 128
EPP = E_CORE // P  # 625 edges per partition
NEG_SLOPE = 0.2
F32 = mybir.dt.float32
MULT = mybir.AluOpType.mult
ADD = mybir.AluOpType.add


def _bcast_free(ap: bass.AP, count: int, axis: int) -> bass.AP:
    """Insert a stride-0 free dim of `count` at free-axis position `axis`."""
    dims = list(ap.ap)
    dims.insert(1 + axis, [0, count])
    return bass.AP(tensor=ap.tensor, offset=ap.offset, ap=dims)


def build_program(
    epp: int = EPP,
    c: int = 25,
    bufs: int = 6,
    q1: int = 7,
    q2: int = 9,
    la: int = 3,
) -> bass.Bass:
    """Build the per-core Bass program for `epp` edges per partition."""
    assert epp % c == 0 and q1 + q2 < c
    t_tiles = epp // c
    e_core = P * epp
    q3 = c - q1 - q2
    q12 = q1 + q2

    nc = bacc.Bacc()
    wi_d = nc.dram_tensor("Wx_i", [e_core, D], F32, kind="ExternalInput")
    wj_d = nc.dram_tensor("Wx_j", [e_core, D], F32, kind="ExternalInput")
    a_d = nc.dram_tensor("a", [REC], F32, kind="ExternalInput")
    out_d = nc.dram_tensor("out", [e_core], F32, kind="ExternalOutput")

    wi = wi_d[:].rearrange("(p n) d -> p n d", p=P)  # [128, epp, 128]
    wj = wj_d[:].rearrange("(p n) d -> p n d", p=P)
    out_r = out_d[:].rearrange("(p n) -> p n", p=P)  # [128, epp]

    with tile.TileContext(nc) as tc, ExitStack() as ctx:
        const_pool = ctx.enter_context(tc.tile_pool(name="const", bufs=1))
        in_pool = ctx.enter_context(tc.tile_pool(name="inp", bufs=bufs))
        pv_pool = ctx.enter_context(tc.tile_pool(name="pv", bufs=2))
        pg_pool = ctx.enter_context(tc.tile_pool(name="pg", bufs=2))
        acc_pool = ctx.enter_context(tc.tile_pool(name="acc", bufs=6))
        res_pool = ctx.enter_context(tc.tile_pool(name="res", bufs=1))

        # attention vector broadcast to all 128 partitions: [128, 2, 128]
        a_sb = const_pool.tile([P, 2, D], F32)
        a_ap = a_d[:]
        a_bcast = bass.AP(
            tensor=a_ap.tensor, offset=a_ap.offset, ap=[[0, P]] + list(a_ap.ap)
        )
        nc.gpsimd.dma_start(out=a_sb[:].rearrange("p a d -> p (a d)"), in_=a_bcast)

        # stride-0 garbage sink for the elementwise out of ScalarE accums
        sink_s = const_pool.tile([P, 1], F32)

        res = res_pool.tile([P, epp], F32)

        # loads are emitted `la` tiles ahead of compute
        recs: dict[int, object] = {}

        def emit_loads(t: int) -> None:
            rec = in_pool.tile([P, 2, c, D], F32, tag="rec")
            recs[t] = rec
            nc.sync.dma_start(out=rec[:, 0, :, :], in_=wi[:, t * c : (t + 1) * c, :])
            nc.sync.dma_start(out=rec[:, 1, :, :], in_=wj[:, t * c : (t + 1) * c, :])

        for t in range(min(la, t_tiles)):
            emit_loads(t)

        for t in range(t_tiles):
            if t + la < t_tiles:
                emit_loads(t + la)
            rec = recs.pop(t)

            # ---- VectorE mult for chains 1+2 (edges [0, q12))
            prod_v = pv_pool.tile([P, 2, q12, D], F32, tag="pv")
            nc.vector.tensor_tensor(
                out=prod_v[:],
                in0=rec[:, :, 0:q12, :],
                in1=_bcast_free(a_sb[:], q12, axis=1),
                op=MULT,
            )
            # ---- GPSIMD mult for chain 3 (edges [q12, c))
            prod_g = pg_pool.tile([P, 2, q3, D], F32, tag="pg")
            nc.gpsimd.tensor_tensor(
                out=prod_g[:],
                in0=rec[:, :, q12:c, :],
                in1=_bcast_free(a_sb[:], q3, axis=1),
                op=MULT,
            )

            # ---- chain 1 reduce on VectorE
            acc_v = acc_pool.tile([P, q1], F32, tag="acc_v")
            nc.vector.tensor_reduce(
                out=acc_v[:],
                in_=prod_v[:, :, 0:q1, :].rearrange("p m c d -> p c m d"),
                axis=mybir.AxisListType.XY,
                op=ADD,
            )
            # ---- chain 3 reduce on VectorE
            acc_g = acc_pool.tile([P, q3], F32, tag="acc_g")
            nc.vector.tensor_reduce(
                out=acc_g[:],
                in_=prod_g[:].rearrange("p m c d -> p c m d"),
                axis=mybir.AxisListType.XY,
                op=ADD,
            )

            # ---- chain 2: ScalarE accumulates per edge from prod_v
            acc_a = acc_pool.tile([P, q2], F32, tag="acc_a")
            for cc in range(q2):
                in_ = prod_v[:, :, q1 + cc, :]
                nc.scalar.activation(
                    out=sink_s[:].broadcast_to(in_.shape),
                    in_=in_,
                    func=mybir.ActivationFunctionType.Copy,
                    accum_out=acc_a[:, cc : cc + 1],
                )

            # leaky relus on ScalarE into the result buffer
            base = t * c
            for accx, lo, hi in (
                (acc_v, 0, q1),
                (acc_a, q1, q12),
                (acc_g, q12, c),
            ):
                nc.scalar.activation(
                    out=res[:, base + lo : base + hi],
                    in_=accx[:],
                    func=mybir.ActivationFunctionType.Prelu,
                    alpha=NEG_SLOPE,
                )

        nc.scalar.dma_start(out=out_r[:, :], in_=res[:])

    nc.compile()
    return nc


_CACHED_NC = None


def kernel(Wx_i: np.ndarray, Wx_j: np.ndarray, a: np.ndarray) -> np.ndarray:
    global _CACHED_NC
    if _CACHED_NC is None:
        _CACHED_NC = build_program()
    nc = _CACHED_NC

    Wx_i = np.ascontiguousarray(np.asarray(Wx_i, dtype=np.float32))
    Wx_j = np.ascontiguousarray(np.asarray(Wx_j, dtype=np.float32))
    a = np.ascontiguousarray(np.asarray(a, dtype=np.float32))

    in_maps = []
    for i in range(N_CORES):
        sl = slice(i * E_CORE, (i + 1) * E_CORE)
        in_maps.append(
            {
                "Wx_i": np.ascontiguousarray(Wx_i[sl]),
                "Wx_j": np.ascontiguousarray(Wx_j[sl]),
                "a": a,
            }
        )

    r = run_bass_kernel_spmd(nc, in_maps, core_ids=list(range(N_CORES)))
    return np.concatenate([m["out"] for m in r.results])
